# revision 47
# baseline (speedup 1.0000x reference)
"""GAT-style multi-head attention (dense adjacency) on 8 TRN2 NeuronCores.

Reference computation:
    h = x @ W.T                       [n, H, d]
    s = h . a_src ; t = h . a_dst     [n, H]
    e[i,j,h] = leaky_relu(s[i,h] + t[j,h], 0.2)
    alpha = softmax_j(where(mask[i,j], e, -inf))
    out[i] = sum_j alpha[i,j,:] h[j]  -> [n, H*d]

Kernel decomposition (per core, core owns a 384-row block of destinations i):
    exp(leaky(e)) = exp(0.2 e) * exp(0.8 relu(e))
                  = exp(0.2 s_i) * exp(0.2 t_j + 0.8 relu(e))
    The exp(0.2 s_i) factor is constant per output row -> cancels in softmax.
    r[j,i]  = max(s_i + 1.25 t_j, 0.25 t_j)          (DVE tensor_scalar, 1 op)
    q       = exp(0.8 r - 3)                         (ACT Exp; -3 also cancels)
    p       = q * mask[j,i]                          (DVE tensor_tensor)
    num/den accumulate via PE matmul with h augmented by a ones column;
    out = num / den.
Scores use layout [j=partition, i=free] so p tiles feed the PE directly as
the stationary operand (fp16 -> fast weight load).
"""

import sys

sys.path.insert(0, "/opt/trn_rl_repo")

from contextlib import ExitStack

import numpy as np

import concourse.bacc as bacc
import concourse.bass as bass
import concourse.mybir as mybir
import concourse.tile as tile
from concourse.bass import ts
from concourse.bass_utils import run_bass_kernel_spmd
from concourse.masks import make_identity

N, F, H, D = 3072, 512, 8, 64
M = 8  # cores
NB = N // M  # 384 destination rows per core
P = 128
NT = N // P  # 24 row tiles
KT = F // P  # 4 contraction tiles
IT = NB // P  # 3 i-subtiles per core
WC = F + 2 * H  # wcat columns: 512 h | 8 Ws_dst | 8 0.8*Ws_src

f32 = mybir.dt.float32
f32r = mybir.dt.float32r
f16 = mybir.dt.float16

TRACE = False
LAST_EXEC_NS = None
LAST_RESULTS = None

_cache = {}


def _build_program(debug_dump=False):
    nc = bacc.Bacc("TRN2", target_bir_lowering=False, debug=False, num_devices=M)
    xT_d = nc.dram_tensor("xT", [F, N], f16, kind="ExternalInput").ap()
    xcT_d = nc.dram_tensor("xcT", [F, NB], f16, kind="ExternalInput").ap()
    wcat_d = nc.dram_tensor("wcat", [F, WC], f16, kind="ExternalInput").ap()
    mask_d = nc.dram_tensor("mask", [N, NB], f16, kind="ExternalInput").ap()
    out_d = nc.dram_tensor("out", [NB, F], f32, kind="ExternalOutput").ap()
    if debug_dump:
        dbg_h = nc.dram_tensor("dbg_h", [P, NT, H, D + 1], f16, kind="ExternalOutput").ap()
        dbg_acc = nc.dram_tensor("dbg_acc", [D + 1, H, NB], f32, kind="ExternalOutput").ap()

    add = mybir.AluOpType.add
    amax = mybir.AluOpType.max
    mult = mybir.AluOpType.mult

    with ExitStack() as ctx:
        tc = ctx.enter_context(tile.TileContext(nc))
        const = ctx.enter_context(tc.tile_pool(name="const", bufs=1))
        dram = ctx.enter_context(tc.tile_pool(name="dram", bufs=1, space="DRAM"))
        xpool = ctx.enter_context(tc.tile_pool(name="xpool", bufs=3))
        pppool = ctx.enter_context(tc.tile_pool(name="pppool", bufs=3))
        ppool = ctx.enter_context(tc.tile_pool(name="ppool", bufs=4))
        spool = ctx.enter_context(tc.tile_pool(name="spool", bufs=8))
        psum_ctx = ExitStack()
        ph_pool = psum_ctx.enter_context(tc.tile_pool(name="ph", bufs=2, space="PSUM"))
        pt_pool = psum_ctx.enter_context(tc.tile_pool(name="pt", bufs=1, space="PSUM"))

        # ---- persistent SBUF ----
        wcat_sb = const.tile([P, KT, WC], f16)
        nc.sync.dma_start(
            out=wcat_sb, in_=wcat_d.rearrange("(kt p) c -> p kt c", p=P)
        )
        xc_sb = const.tile([P, KT, NB], f16)
        nc.sync.dma_start(
            out=xc_sb, in_=xcT_d.rearrange("(kt p) i -> p kt i", p=P)
        )
        mask_sb = const.tile([P, NT, NB], f16)
        for c in range(4):
            nc.sync.dma_start(
                out=mask_sb[:, ts(c, 6), :],
                in_=mask_d.rearrange("(jt p) i -> p jt i", p=P)[:, ts(c, 6), :],
            )
        h_aug = const.tile([P, NT, H, D + 1], f16)
        nc.vector.memset(h_aug[:, :, :, D : D + 1], 1.0)
        tsb = const.tile([P, NT, H], f32)  # t - 3 per row tile
        et = const.tile([P, NT, H], f32)  # exp(t - 3)
        ed = const.tile([P, NT, H], f32)  # exp(-0.8 t)
        es_b = const.tile([P, H, NB], f16)  # exp(0.8 s_i), broadcast over partitions
        bias_z = const.tile([P, 1], f32)
        nc.vector.memset(bias_z, 0.0)
        bias_m3 = const.tile([P, 1], f32)
        nc.vector.memset(bias_m3, -3.0)
        bias_m24 = const.tile([P, 1], f32)
        nc.vector.memset(bias_m24, -2.4)

        # ---- s path: es[h, i] = exp(0.8 s) for this core's i block,
        # broadcast to all partitions via a DRAM bounce
        psum_s = ph_pool.tile([H, NB], f32, tag="ph")
        for kt in range(KT):
            nc.tensor.matmul(
                psum_s,
                lhsT=wcat_sb[:, kt, F + H : F + 2 * H],
                rhs=xc_sb[:, kt, :],
                start=(kt == 0),
                stop=(kt == KT - 1),
            )
        s_sb = const.tile([H, NB], f16)
        nc.scalar.activation(
            out=s_sb,
            in_=psum_s,
            func=mybir.ActivationFunctionType.Exp,
            bias=bias_z[0:H],
        )
        s_dram = dram.tile([H, NB], f16)
        nc.sync.dma_start(out=s_dram, in_=s_sb)
        for hh in range(H):
            nc.sync.dma_start(
                out=es_b[:, hh, :], in_=s_dram[hh, :].partition_broadcast(P)
            )

        # ---- stage 0: h (fp16, augmented) and t columns for every row tile
        for mt in range(NT):
            xt_sb = xpool.tile([P, KT, P], f16, tag="xt")
            nc.sync.dma_start(
                out=xt_sb,
                in_=xT_d[:, ts(mt, P)].rearrange("(kt p) m -> p kt m", p=P),
            )
            psum_h = ph_pool.tile([P, F], f32, tag="ph")
            psum_t = pt_pool.tile([P, H], f32, tag="pt")
            for kt in range(KT):
                nc.tensor.matmul(
                    psum_h,
                    lhsT=xt_sb[:, kt, :],
                    rhs=wcat_sb[:, kt, 0:F],
                    start=(kt == 0),
                    stop=(kt == KT - 1),
                )
                nc.tensor.matmul(
                    psum_t,
                    lhsT=xt_sb[:, kt, :],
                    rhs=wcat_sb[:, kt, F : F + H],
                    start=(kt == 0),
                    stop=(kt == KT - 1),
                )
            nc.scalar.activation(
                out=h_aug[:, mt, :, 0:D],
                in_=psum_h.rearrange("p (h d) -> p h d", h=H),
                func=mybir.ActivationFunctionType.Copy,
            )
            nc.scalar.activation(
                out=tsb[:, mt, :],
                in_=psum_t,
                func=mybir.ActivationFunctionType.Identity,
                bias=bias_m3,
            )
            if mt % 6 == 5:
                ch = ts(mt // 6, 6)
                nc.scalar.activation(
                    out=et[:, ch, :],
                    in_=tsb[:, ch, :],
                    func=mybir.ActivationFunctionType.Exp,
                    bias=bias_z,
                )
                nc.scalar.activation(
                    out=ed[:, ch, :],
                    in_=tsb[:, ch, :],
                    func=mybir.ActivationFunctionType.Exp,
                    bias=bias_m24,
                    scale=-0.8,
                )

        identity = const.tile([P, P], f32)
        make_identity(nc, identity)
        outf = const.tile([P, IT, F], f32)
        if debug_dump:
            nc.sync.dma_start(out=dbg_h, in_=h_aug)

        # Release stage-0 PSUM pools so the 8 per-head accumulators below can
        # each own a full PSUM bank (matmul start=True clears whole banks, so
        # accumulators must not share banks with anything).
        psum_ctx.close()

        # ---- main loop: one accumulator bank per head ([65, 384] accT)
        o_sb8 = const.tile([D + 1, H, NB], f32)
        with tc.tile_pool(name="acc", bufs=1, space="PSUM") as acc_pool:
            accs = [acc_pool.tile([D + 1, NB], f32, name=f"acc{a}") for a in range(H)]
            zero11 = const.tile([1, 1], f16)
            nc.vector.memset(zero11, 0.0)
            HH = H // 2
            for jt in range(NT):
                pp_t = pppool.tile([P, H, NB], f16, tag="pp")
                for hh in range(H):
                    # p' = et_j * max(es_i, ed_j)  (= max(exp(.8s+t-3), exp(.2t-3)))
                    nc.vector.tensor_scalar(
                        out=pp_t[:, hh, :],
                        in0=es_b[:, hh, :],
                        scalar1=ed[:, jt, hh : hh + 1],
                        scalar2=et[:, jt, hh : hh + 1],
                        op0=amax,
                        op1=mult,
                    )
                    if jt > 0 and hh in (2, 5):
                        # keep-alive: 0-matmul so the PE never idles past the
                        # HAM window between per-jt matmul bursts
                        nc.tensor.matmul(
                            accs[0][0:1, 0:1],
                            lhsT=zero11,
                            rhs=pp_t[0:1, hh, 0:1],
                            start=False,
                            stop=False,
                            skip_group_check=True,
                        )
                p_t = ppool.tile([P, H, NB], f16, tag="p")
                for g2 in range(2):
                    nc.vector.tensor_tensor(
                        out=p_t[:, ts(g2, HH), :],
                        in0=pp_t[:, ts(g2, HH), :],
                        in1=mask_sb[:, jt, :].unsqueeze(1).broadcast_to((P, HH, NB)),
                        op=mult,
                    )
                    for hl in range(HH):
                        hh = g2 * HH + hl
                        nc.tensor.matmul(
                            accs[hh],
                            lhsT=h_aug[:, jt, hh, :],
                            rhs=p_t[:, hh, :],
                            start=(jt == 0),
                            stop=(jt == NT - 1),
                            skip_group_check=True,
                        )

            for hh in range(H):
                nc.scalar.activation(
                    out=o_sb8[:, hh, :],
                    in_=accs[hh],
                    func=mybir.ActivationFunctionType.Copy,
                )

        if debug_dump:
            nc.sync.dma_start(out=dbg_acc, in_=o_sb8)

        # ---- epilogue: transpose accT back to [i, d], divide by den
        with tc.tile_pool(name="tr", bufs=2, space="PSUM") as tr_pool:
            for hh in range(H):
                for it in range(IT):
                    tr = tr_pool.tile([P, D + 1], f32, tag="tr")
                    nc.tensor.transpose(
                        tr, o_sb8[:, hh, ts(it, P)], identity[0 : D + 1, 0 : D + 1]
                    )
                    rec = spool.tile([P, 1], f32, tag="rec")
                    nc.vector.reciprocal(rec, tr[:, D : D + 1])
                    nc.scalar.activation(
                        out=outf[:, it, ts(hh, D)],
                        in_=tr[:, 0:D],
                        func=mybir.ActivationFunctionType.Identity,
                        bias=bias_z,
                        scale=rec,
                    )

        for it in range(IT):
            nc.sync.dma_start(out=out_d[ts(it, P), :], in_=outf[:, it, :])

    nc.compile()
    return nc


def _sim_check(in_map, debug_dump=False):
    """Run the single-core interpreter against one core's inputs (debug aid)."""
    from concourse.bass_interp import CoreSim

    nc = _build_program(debug_dump=debug_dump)
    sim = CoreSim(nc, trace=False)
    for k, v in in_map.items():
        sim.tensor(k)[:] = v
    sim.simulate()
    names = ["out"] + (["dbg_h", "dbg_acc"] if debug_dump else [])
    return {n: np.array(sim.tensor(n)) for n in names}


def _pack_inputs(x, adj, W, a_src, a_dst):
    x = np.asarray(x, dtype=np.float32)
    adj = np.asarray(adj, dtype=np.int32)
    W = np.asarray(W, dtype=np.float32)
    a_src = np.asarray(a_src, dtype=np.float32)
    a_dst = np.asarray(a_dst, dtype=np.float32)

    xT = np.ascontiguousarray(x.T.astype(np.float16))
    Wr = W.reshape(H, D, F)
    Ws_src = np.einsum("hdf,hd->fh", Wr, a_src).astype(np.float32)
    Ws_dst = np.einsum("hdf,hd->fh", Wr, a_dst).astype(np.float32)
    wcat = np.concatenate(
        [np.ascontiguousarray(W.T), Ws_dst, 0.8 * Ws_src], axis=1
    ).astype(np.float16)
    mask = (adj + np.eye(N, dtype=np.int32)) > 0  # [i, j]
    in_maps = []
    for c in range(M):
        mask_c = np.ascontiguousarray(
            mask[c * NB : (c + 1) * NB, :].T.astype(np.float16)
        )  # [j, i]
        xcT = np.ascontiguousarray(xT[:, c * NB : (c + 1) * NB])
        in_maps.append({"xT": xT, "xcT": xcT, "wcat": wcat, "mask": mask_c})
    return in_maps


def _install_ntff_hook():
    """Recreate antenv.axon_hooks (absent in this image) so that
    run_bass_kernel_spmd(trace=True) can capture NTFF profiles through
    the axon PJRT .so. Degrades silently when unavailable."""
    import contextlib
    import ctypes
    import os
    import types

    try:
        from antenv.axon_hooks import get_axon_ntff_profile_hook  # noqa: F401

        return True
    except ImportError:
        pass
    so_path = "/opt/axon/libaxon_pjrt.so"
    if not os.path.exists(so_path):
        return False
    lib = ctypes.CDLL(so_path)
    if not hasattr(lib, "axon_start_nrt_profile"):
        return False
    lib.axon_start_nrt_profile.argtypes = [
        ctypes.POINTER(ctypes.c_int64),
        ctypes.c_size_t,
    ]
    lib.axon_start_nrt_profile.restype = ctypes.c_int64
    lib.axon_stop_nrt_profile.argtypes = [ctypes.c_char_p]
    lib.axon_stop_nrt_profile.restype = ctypes.c_int64

    @contextlib.contextmanager
    def _hook(output_dir, device_ids):
        import jax

        jax.devices()
        if device_ids:
            ids = (ctypes.c_int64 * len(device_ids))(*device_ids)
            rc = lib.axon_start_nrt_profile(ids, len(device_ids))
        else:
            rc = lib.axon_start_nrt_profile(None, 0)
        if rc != 0:
            raise RuntimeError(f"axon_start_nrt_profile rc={rc}")
        try:
            yield
        finally:
            n = lib.axon_stop_nrt_profile(str(output_dir).encode())
            print(f"ntff profile: {n} file(s) written to {output_dir}")

    mod = types.ModuleType("antenv.axon_hooks")
    _state = {"hook": _hook}
    mod.get_axon_ntff_profile_hook = lambda: _state["hook"]
    mod.set_axon_ntff_profile_hook = lambda h: _state.__setitem__("hook", h)
    import antenv

    antenv.axon_hooks = mod
    sys.modules["antenv.axon_hooks"] = mod
    return True


def kernel(x, adj, W, a_src, a_dst):
    global LAST_EXEC_NS, LAST_RESULTS
    if "nc" not in _cache:
        _cache["nc"] = _build_program()
    nc = _cache["nc"]
    if TRACE:
        _install_ntff_hook()
    in_maps = _pack_inputs(x, adj, W, a_src, a_dst)
    res = run_bass_kernel_spmd(nc, in_maps, core_ids=list(range(M)), trace=TRACE)
    LAST_EXEC_NS = res.exec_time_ns
    LAST_RESULTS = res
    out = np.concatenate([res.results[c]["out"] for c in range(M)], axis=0)
    return out.astype(np.float32)


# revision 56
# speedup vs baseline: 1.0928x; 1.0928x over previous
"""GAT-style multi-head attention (dense adjacency) on 8 TRN2 NeuronCores.

Reference computation:
    h = x @ W.T                       [n, H, d]
    s = h . a_src ; t = h . a_dst     [n, H]
    e[i,j,h] = leaky_relu(s[i,h] + t[j,h], 0.2)
    alpha = softmax_j(where(mask[i,j], e, -inf))
    out[i] = sum_j alpha[i,j,:] h[j]  -> [n, H*d]

Kernel decomposition (per core, core owns a 384-row block of destinations i):
    exp(leaky(e)) = exp(0.2 e) * exp(0.8 relu(e))
                  = exp(0.2 s_i) * exp(0.2 t_j + 0.8 relu(e))
    The exp(0.2 s_i) factor is constant per output row -> cancels in softmax.
    r[j,i]  = max(s_i + 1.25 t_j, 0.25 t_j)          (DVE tensor_scalar, 1 op)
    q       = exp(0.8 r - 3)                         (ACT Exp; -3 also cancels)
    p       = q * mask[j,i]                          (DVE tensor_tensor)
    num/den accumulate via PE matmul with h augmented by a ones column;
    out = num / den.
Scores use layout [j=partition, i=free] so p tiles feed the PE directly as
the stationary operand (fp16 -> fast weight load).
"""

import sys

sys.path.insert(0, "/opt/trn_rl_repo")

from contextlib import ExitStack

import numpy as np

import concourse.bacc as bacc
import concourse.bass as bass
import concourse.mybir as mybir
import concourse.tile as tile
from concourse.bass import ts
from concourse.bass_utils import run_bass_kernel_spmd
from concourse.masks import make_identity

N, F, H, D = 3072, 512, 8, 64
M = 8  # cores
NB = N // M  # 384 destination rows per core
P = 128
NT = N // P  # 24 row tiles
KT = F // P  # 4 contraction tiles
IT = NB // P  # 3 i-subtiles per core
WC = F + 2 * H  # wcat columns: 512 h | 8 Ws_dst | 8 0.8*Ws_src

f32 = mybir.dt.float32
f32r = mybir.dt.float32r
f16 = mybir.dt.float16

TRACE = False
LAST_EXEC_NS = None
LAST_RESULTS = None

_cache = {}


def _build_program(debug_dump=False):
    nc = bacc.Bacc("TRN2", target_bir_lowering=False, debug=False, num_devices=M)
    xT_d = nc.dram_tensor("xT", [F, N], f16, kind="ExternalInput").ap()
    xcT_d = nc.dram_tensor("xcT", [F, NB], f16, kind="ExternalInput").ap()
    wcat_d = nc.dram_tensor("wcat", [F, WC], f16, kind="ExternalInput").ap()
    mask_d = nc.dram_tensor("mask", [N, NB], f16, kind="ExternalInput").ap()
    out_d = nc.dram_tensor("out", [NB, F], f32, kind="ExternalOutput").ap()
    if debug_dump:
        dbg_h = nc.dram_tensor("dbg_h", [P, NT, H, D + 1], f16, kind="ExternalOutput").ap()
        dbg_acc = nc.dram_tensor("dbg_acc", [D + 1, H, NB], f32, kind="ExternalOutput").ap()

    add = mybir.AluOpType.add
    amax = mybir.AluOpType.max
    mult = mybir.AluOpType.mult

    with ExitStack() as ctx:
        tc = ctx.enter_context(tile.TileContext(nc))
        const = ctx.enter_context(tc.tile_pool(name="const", bufs=1))
        dram = ctx.enter_context(tc.tile_pool(name="dram", bufs=1, space="DRAM"))
        xpool = ctx.enter_context(tc.tile_pool(name="xpool", bufs=3))
        pppool = ctx.enter_context(tc.tile_pool(name="pppool", bufs=4))
        ppool = ctx.enter_context(tc.tile_pool(name="ppool", bufs=6))
        spool = ctx.enter_context(tc.tile_pool(name="spool", bufs=8))
        psum_ctx = ExitStack()
        ph_pool = psum_ctx.enter_context(tc.tile_pool(name="ph", bufs=2, space="PSUM"))
        pt_pool = psum_ctx.enter_context(tc.tile_pool(name="pt", bufs=1, space="PSUM"))

        # ---- persistent SBUF ----
        wcat_sb = const.tile([P, KT, WC], f16)
        nc.sync.dma_start(
            out=wcat_sb, in_=wcat_d.rearrange("(kt p) c -> p kt c", p=P)
        )
        xc_sb = const.tile([P, KT, NB], f16)
        nc.sync.dma_start(
            out=xc_sb, in_=xcT_d.rearrange("(kt p) i -> p kt i", p=P)
        )
        mask_sb = const.tile([P, NT, NB], f16)
        h_aug = const.tile([P, NT, H, D + 1], f16)
        nc.vector.memset(h_aug[:, :, :, D : D + 1], 1.0)
        tsb = const.tile([P, NT, H], f32)  # t - 3 per row tile
        et = const.tile([P, NT, H], f32)  # exp(t - 3)
        ed = const.tile([P, NT, H], f32)  # exp(-0.8 t)
        es_b = const.tile([P, H, NB], f16)  # exp(0.8 s_i), broadcast over partitions
        bias_z = const.tile([P, 1], f32)
        nc.vector.memset(bias_z, 0.0)
        bias_m3 = const.tile([P, 1], f32)
        nc.vector.memset(bias_m3, -3.0)
        bias_m24 = const.tile([P, 1], f32)
        nc.vector.memset(bias_m24, -2.4)

        # ---- s path: es[h, i] = exp(0.8 s) for this core's i block,
        # broadcast to all partitions via a DRAM bounce
        psum_s = ph_pool.tile([H, NB], f32, tag="ph")
        for kt in range(KT):
            nc.tensor.matmul(
                psum_s,
                lhsT=wcat_sb[:, kt, F + H : F + 2 * H],
                rhs=xc_sb[:, kt, :],
                start=(kt == 0),
                stop=(kt == KT - 1),
            )
        s_sb = const.tile([H, NB], f16)
        nc.scalar.activation(
            out=s_sb,
            in_=psum_s,
            func=mybir.ActivationFunctionType.Exp,
            bias=bias_z[0:H],
        )
        sel = const.tile([H, H, P], f16)
        nc.gpsimd.memset(sel, 0.0)
        # sel[k, h, :] = 1 where k == h (row-selector matrices)
        nc.gpsimd.affine_select(
            out=sel,
            in_=sel,
            compare_op=mybir.AluOpType.not_equal,
            fill=1.0,
            base=0,
            pattern=[[-1, H], [0, P]],
            channel_multiplier=1,
        )
        for hh in range(H):
            psum_b = ph_pool.tile([P, NB], f32, tag="ph")
            nc.tensor.matmul(psum_b, lhsT=sel[:, hh, :], rhs=s_sb)
            nc.scalar.activation(
                out=es_b[:, hh, :],
                in_=psum_b,
                func=mybir.ActivationFunctionType.Copy,
            )

        # ---- stage 0: h (fp16, augmented) and t columns for every row tile
        for mt in range(NT):
            xt_sb = xpool.tile([P, KT, P], f16, tag="xt")
            nc.sync.dma_start(
                out=xt_sb,
                in_=xT_d[:, ts(mt, P)].rearrange("(kt p) m -> p kt m", p=P),
            )
            psum_h = ph_pool.tile([P, F], f32, tag="ph")
            psum_t = pt_pool.tile([P, H], f32, tag="pt")
            for kt in range(KT):
                nc.tensor.matmul(
                    psum_h,
                    lhsT=xt_sb[:, kt, :],
                    rhs=wcat_sb[:, kt, 0:F],
                    start=(kt == 0),
                    stop=(kt == KT - 1),
                )
                nc.tensor.matmul(
                    psum_t,
                    lhsT=xt_sb[:, kt, :],
                    rhs=wcat_sb[:, kt, F : F + H],
                    start=(kt == 0),
                    stop=(kt == KT - 1),
                )
            nc.scalar.activation(
                out=h_aug[:, mt, :, 0:D],
                in_=psum_h.rearrange("p (h d) -> p h d", h=H),
                func=mybir.ActivationFunctionType.Copy,
            )
            nc.scalar.activation(
                out=tsb[:, mt, :],
                in_=psum_t,
                func=mybir.ActivationFunctionType.Identity,
                bias=bias_m3,
            )
            if mt % 3 == 2:
                ch = ts(mt // 3, 3)
                nc.scalar.activation(
                    out=et[:, ch, :],
                    in_=tsb[:, ch, :],
                    func=mybir.ActivationFunctionType.Exp,
                    bias=bias_z,
                )
                nc.scalar.activation(
                    out=ed[:, ch, :],
                    in_=tsb[:, ch, :],
                    func=mybir.ActivationFunctionType.Exp,
                    bias=bias_m24,
                    scale=-0.8,
                )

        for c in range(8):
            nc.sync.dma_start(
                out=mask_sb[:, ts(c, 3), :],
                in_=mask_d.rearrange("(jt p) i -> p jt i", p=P)[:, ts(c, 3), :],
            )
        identity = const.tile([P, P], f32)
        make_identity(nc, identity)
        outf = const.tile([P, IT, F], f32)
        if debug_dump:
            nc.sync.dma_start(out=dbg_h, in_=h_aug)

        # Release stage-0 PSUM pools so the 8 per-head accumulators below can
        # each own a full PSUM bank (matmul start=True clears whole banks, so
        # accumulators must not share banks with anything).
        psum_ctx.close()

        # ---- main loop: one accumulator bank per head ([65, 384] accT)
        o_sb8 = const.tile([D + 1, H, NB], f32)
        with tc.tile_pool(name="acc", bufs=1, space="PSUM") as acc_pool:
            accs = [acc_pool.tile([D + 1, NB], f32, name=f"acc{a}") for a in range(H)]
            zero11 = const.tile([1, 1], f16)
            nc.vector.memset(zero11, 0.0)
            HH = H // 2
            for jt in range(NT):
                pp_t = pppool.tile([P, H, NB], f16, tag="pp")
                for hh in range(H):
                    # p' = et_j * max(es_i, ed_j)  (= max(exp(.8s+t-3), exp(.2t-3)))
                    nc.vector.tensor_scalar(
                        out=pp_t[:, hh, :],
                        in0=es_b[:, hh, :],
                        scalar1=ed[:, jt, hh : hh + 1],
                        scalar2=et[:, jt, hh : hh + 1],
                        op0=amax,
                        op1=mult,
                    )
                    if jt > 0 and hh in (2, 5):
                        # keep-alive: 0-matmul so the PE never idles past the
                        # HAM window between per-jt matmul bursts
                        nc.tensor.matmul(
                            accs[0][0:1, 0:1],
                            lhsT=zero11,
                            rhs=pp_t[0:1, hh, 0:1],
                            start=False,
                            stop=False,
                            skip_group_check=True,
                        )
                p_t = ppool.tile([P, H, NB], f16, tag="p")
                for g2 in range(2):
                    nc.vector.tensor_tensor(
                        out=p_t[:, ts(g2, HH), :],
                        in0=pp_t[:, ts(g2, HH), :],
                        in1=mask_sb[:, jt, :].unsqueeze(1).broadcast_to((P, HH, NB)),
                        op=mult,
                    )
                    for hl in range(HH):
                        hh = g2 * HH + hl
                        nc.tensor.matmul(
                            accs[hh],
                            lhsT=h_aug[:, jt, hh, :],
                            rhs=p_t[:, hh, :],
                            start=(jt == 0),
                            stop=(jt == NT - 1),
                            skip_group_check=True,
                        )

            for hh in range(H):
                nc.scalar.activation(
                    out=o_sb8[:, hh, :],
                    in_=accs[hh],
                    func=mybir.ActivationFunctionType.Copy,
                )

        if debug_dump:
            nc.sync.dma_start(out=dbg_acc, in_=o_sb8)

        # ---- epilogue: transpose accT back to [i, d], divide by den
        with tc.tile_pool(name="tr", bufs=2, space="PSUM") as tr_pool:
            for hh in range(H):
                for it in range(IT):
                    tr = tr_pool.tile([P, D + 1], f32, tag="tr")
                    nc.tensor.transpose(
                        tr, o_sb8[:, hh, ts(it, P)], identity[0 : D + 1, 0 : D + 1]
                    )
                    rec = spool.tile([P, 1], f32, tag="rec")
                    nc.vector.reciprocal(rec, tr[:, D : D + 1])
                    nc.scalar.activation(
                        out=outf[:, it, ts(hh, D)],
                        in_=tr[:, 0:D],
                        func=mybir.ActivationFunctionType.Identity,
                        bias=bias_z,
                        scale=rec,
                    )

        for it in range(IT):
            nc.sync.dma_start(out=out_d[ts(it, P), :], in_=outf[:, it, :])

    nc.compile()
    return nc


def _sim_check(in_map, debug_dump=False):
    """Run the single-core interpreter against one core's inputs (debug aid)."""
    from concourse.bass_interp import CoreSim

    nc = _build_program(debug_dump=debug_dump)
    sim = CoreSim(nc, trace=False)
    for k, v in in_map.items():
        sim.tensor(k)[:] = v
    sim.simulate()
    names = ["out"] + (["dbg_h", "dbg_acc"] if debug_dump else [])
    return {n: np.array(sim.tensor(n)) for n in names}


def _pack_inputs(x, adj, W, a_src, a_dst):
    x = np.asarray(x, dtype=np.float32)
    adj = np.asarray(adj, dtype=np.int32)
    W = np.asarray(W, dtype=np.float32)
    a_src = np.asarray(a_src, dtype=np.float32)
    a_dst = np.asarray(a_dst, dtype=np.float32)

    xT = np.ascontiguousarray(x.T.astype(np.float16))
    Wr = W.reshape(H, D, F)
    Ws_src = np.einsum("hdf,hd->fh", Wr, a_src).astype(np.float32)
    Ws_dst = np.einsum("hdf,hd->fh", Wr, a_dst).astype(np.float32)
    wcat = np.concatenate(
        [np.ascontiguousarray(W.T), Ws_dst, 0.8 * Ws_src], axis=1
    ).astype(np.float16)
    mask = (adj + np.eye(N, dtype=np.int32)) > 0  # [i, j]
    in_maps = []
    for c in range(M):
        mask_c = np.ascontiguousarray(
            mask[c * NB : (c + 1) * NB, :].T.astype(np.float16)
        )  # [j, i]
        xcT = np.ascontiguousarray(xT[:, c * NB : (c + 1) * NB])
        in_maps.append({"xT": xT, "xcT": xcT, "wcat": wcat, "mask": mask_c})
    return in_maps


def _install_ntff_hook():
    """Recreate antenv.axon_hooks (absent in this image) so that
    run_bass_kernel_spmd(trace=True) can capture NTFF profiles through
    the axon PJRT .so. Degrades silently when unavailable."""
    import contextlib
    import ctypes
    import os
    import types

    try:
        from antenv.axon_hooks import get_axon_ntff_profile_hook  # noqa: F401

        return True
    except ImportError:
        pass
    so_path = "/opt/axon/libaxon_pjrt.so"
    if not os.path.exists(so_path):
        return False
    lib = ctypes.CDLL(so_path)
    if not hasattr(lib, "axon_start_nrt_profile"):
        return False
    lib.axon_start_nrt_profile.argtypes = [
        ctypes.POINTER(ctypes.c_int64),
        ctypes.c_size_t,
    ]
    lib.axon_start_nrt_profile.restype = ctypes.c_int64
    lib.axon_stop_nrt_profile.argtypes = [ctypes.c_char_p]
    lib.axon_stop_nrt_profile.restype = ctypes.c_int64

    @contextlib.contextmanager
    def _hook(output_dir, device_ids):
        import jax

        jax.devices()
        if device_ids:
            ids = (ctypes.c_int64 * len(device_ids))(*device_ids)
            rc = lib.axon_start_nrt_profile(ids, len(device_ids))
        else:
            rc = lib.axon_start_nrt_profile(None, 0)
        if rc != 0:
            raise RuntimeError(f"axon_start_nrt_profile rc={rc}")
        try:
            yield
        finally:
            n = lib.axon_stop_nrt_profile(str(output_dir).encode())
            print(f"ntff profile: {n} file(s) written to {output_dir}")

    mod = types.ModuleType("antenv.axon_hooks")
    _state = {"hook": _hook}
    mod.get_axon_ntff_profile_hook = lambda: _state["hook"]
    mod.set_axon_ntff_profile_hook = lambda h: _state.__setitem__("hook", h)
    import antenv

    antenv.axon_hooks = mod
    sys.modules["antenv.axon_hooks"] = mod
    return True


def kernel(x, adj, W, a_src, a_dst):
    global LAST_EXEC_NS, LAST_RESULTS
    if "nc" not in _cache:
        _cache["nc"] = _build_program()
    nc = _cache["nc"]
    if TRACE:
        _install_ntff_hook()
    in_maps = _pack_inputs(x, adj, W, a_src, a_dst)
    res = run_bass_kernel_spmd(nc, in_maps, core_ids=list(range(M)), trace=TRACE)
    LAST_EXEC_NS = res.exec_time_ns
    LAST_RESULTS = res
    out = np.concatenate([res.results[c]["out"] for c in range(M)], axis=0)
    return out.astype(np.float32)
